# revision 9
# baseline (speedup 1.0000x reference)
"""GQA attention (tanh-score + static bias, no softmax) on 8 trn2 cores.

Reference shapes: x [4,32,256,512], H=8 heads, G=2 kv groups, D=64, N=256.
Strategy: data-parallel over the 128 (b,t) pairs -> 16 per core, zero
collectives.  Per (b,t):
  q = x@Wq, k = x@Wk, v = x@Wv          (feature-major via host-transposed x)
  scores^T[m,n] = k_g @ q_h^T           (K=64 contraction, base-aligned)
  attn^T = tanh(scores^T * 0.125)       (ACT engine, scale fused)
  out_h^T = v_g^T @ attn_h^T + (sgr v_g)^T   (sgr@v once per group, add fused
                                              into the PSUM->SBUF evacuation)
  y = out @ Wo                           (Wo host-permuted to match pair order)

Host-side prep (outside the HW kernel): x transposed to feature-major and
pre-tiled, sgr transposed, Wk concatenated with its group-swapped copy (so
every head's score matmul finds its K block at the right partition offset),
Wo row-permuted.
"""

import os
import sys

import numpy as np

for _p in ("/opt/trn_rl_repo",):
    if _p not in sys.path and os.path.isdir(_p):
        sys.path.insert(0, _p)

import concourse.bass as bass
import concourse.tile as tile
from concourse import bacc, mybir
from concourse.bass_utils import run_bass_kernel_spmd

F32 = mybir.dt.float32
F32R = mybir.dt.float32r

B, T, N, C = 4, 32, 256, 512
H, G, D = 8, 2, 64
NCORES = 8
BT = B * T                      # 128
PER_CORE = BT // NCORES         # 16
NPAIR = PER_CORE // 2           # 8 iterations of 2 (b,t) each
SCALE = D ** -0.5               # 0.125

_cached = {}


def _build_nc():
    """Build + lower the single-core SPMD program."""
    nc = bacc.Bacc("TRN2", target_bir_lowering=False, debug=False,
                   num_devices=NCORES)

    # DRAM I/O (per-core shard, host-side pre-arranged)
    # xarr[i, p, c, 256*b + n] = x[bt=2i+b, tok=n, cin=128c+p]
    xT = nc.dram_tensor("xT", [NPAIR, 128, 4, 512], F32R, kind="ExternalInput").ap()
    sgrT = nc.dram_tensor("sgrT", [N, N], F32R, kind="ExternalInput").ap()
    Wq = nc.dram_tensor("Wq", [C, C], F32R, kind="ExternalInput").ap()
    Wkc = nc.dram_tensor("Wkc", [C, 2 * G * D], F32R, kind="ExternalInput").ap()
    Wv = nc.dram_tensor("Wv", [C, G * D], F32R, kind="ExternalInput").ap()
    Wop = nc.dram_tensor("Wop", [C, C], F32R, kind="ExternalInput").ap()
    y = nc.dram_tensor("y", [PER_CORE, N, C], F32, kind="ExternalOutput").ap()

    with tile.TileContext(nc) as tc:
        _body(tc, xT, sgrT, Wq, Wkc, Wv, Wop, y)

    nc.compile()
    return nc


def _body(tc, xT, sgrT, Wq, Wkc, Wv, Wop, y):
    nc = tc.nc

    def mm(out, lhsT, rhs, **kw):
        # float32r streams 1 row/cycle (vs 4 for fp32) at free dim >= 256.
        # All lhsT/rhs tiles are allocated as F32R so their producers round.
        nc.tensor.matmul(out, lhsT, rhs, **kw)
    import contextlib
    ctx = contextlib.ExitStack()
    with ctx:
        consts = ctx.enter_context(tc.tile_pool(name="consts", bufs=1))
        xpool = ctx.enter_context(tc.tile_pool(name="xt", bufs=2))
        qpool = ctx.enter_context(tc.tile_pool(name="qs", bufs=8))
        kpool = ctx.enter_context(tc.tile_pool(name="ks", bufs=4))
        vpool = ctx.enter_context(tc.tile_pool(name="vs", bufs=8))
        svpool = ctx.enter_context(tc.tile_pool(name="svs", bufs=4))
        apool = ctx.enter_context(tc.tile_pool(name="attn", bufs=6))
        ppool = ctx.enter_context(tc.tile_pool(name="pairs", bufs=10))
        ypool = ctx.enter_context(tc.tile_pool(name="ys", bufs=6))
        psA = ctx.enter_context(
            tc.tile_pool(name="psA", bufs=4, space=bass.MemorySpace.PSUM))
        psB = ctx.enter_context(
            tc.tile_pool(name="psB", bufs=4, space=bass.MemorySpace.PSUM))

        # ---- resident constants ----
        wq = []
        wkc = []
        wv = []
        wo = []
        for c in range(4):
            t = consts.tile([128, 512], F32R, tag=f"wq{c}")
            nc.sync.dma_start(t[:], Wq[128 * c:128 * (c + 1), :])
            wq.append(t)
            t = consts.tile([128, 256], F32R, tag=f"wkc{c}")
            nc.sync.dma_start(t[:], Wkc[128 * c:128 * (c + 1), :])
            wkc.append(t)
            t = consts.tile([128, 128], F32R, tag=f"wv{c}")
            nc.sync.dma_start(t[:], Wv[128 * c:128 * (c + 1), :])
            wv.append(t)
            t = consts.tile([128, 512], F32R, tag=f"wo{c}")
            nc.sync.dma_start(t[:], Wop[128 * c:128 * (c + 1), :])
            wo.append(t)
        sgt = []
        for mc in range(2):
            t = consts.tile([128, 256], F32R, tag=f"sgt{mc}")
            nc.sync.dma_start(t[:], sgrT[128 * mc:128 * (mc + 1), :])
            sgt.append(t)

        # per-iteration state handed from stage A to stage B
        state = [None] * NPAIR

        def stage_a(it):
            xt = xpool.tile([128, 4, 512], F32R, tag="xt")
            nc.sync.dma_start(xt[:], xT[it])

            # q projection: feature-major q^T, couts 128j..128j+127,
            # free = 512 (two bt's 256 tokens each)
            qs = []
            for j in range(4):
                ps = psA.tile([128, 512], F32, tag="psA")
                for c in range(4):
                    mm(ps[:], wq[c][:, 128 * j:128 * (j + 1)],
                                     xt[:, c, :], start=(c == 0), stop=(c == 3))
                s = qpool.tile([128, 512], F32R, tag="qs")
                nc.vector.tensor_copy(s[:], ps[:])
                qs.append(s)

            # k projections: k1 = [g0;g1] rows, k2 = [g1;g0] rows
            ks = []
            for jj in range(2):
                ps = psA.tile([128, 512], F32, tag="psA")
                for c in range(4):
                    mm(ps[:], wkc[c][:, 128 * jj:128 * (jj + 1)],
                                     xt[:, c, :], start=(c == 0), stop=(c == 3))
                s = kpool.tile([128, 512], F32R, tag="ks")
                nc.vector.tensor_copy(s[:], ps[:])
                ks.append(s)

            # v token-major: [tok 128, 128 (g0 d | g1 d)] per (b, tok-chunk)
            vs = [[None, None], [None, None]]
            for b in range(2):
                for mc in range(2):
                    ps = psB.tile([128, 128], F32, tag="psB")
                    off = 256 * b + 128 * mc
                    for c in range(4):
                        mm(ps[:], xt[:, c, off:off + 128],
                                         wv[c][:], start=(c == 0), stop=(c == 3))
                    s = vpool.tile([128, 128], F32R, tag="vs")
                    nc.vector.tensor_copy(s[:], ps[:])
                    vs[b][mc] = s

            # sgr @ v, both groups at once: lhsT = full v tile [m,128] so
            # out rows 0:64 = (sgr v_g0)^T, rows 64:128 = (sgr v_g1)^T and the
            # PSUM dst stays at partition base 0 (fp32r ISA constraint).
            svs = []
            for b in range(2):
                ps = psB.tile([128, 256], F32, tag="psB")
                for mc in range(2):
                    mm(ps[:], vs[b][mc][:], sgt[mc][:],
                       start=(mc == 0), stop=(mc == 1))
                s = svpool.tile([128, 256], F32, tag="svs")
                nc.vector.tensor_copy(s[:], ps[:])
                svs.append(s)

            state[it] = (qs, ks, vs, svs)

        def stage_b(it):
            qs, ks, vs, svs = state[it]
            # scores + tanh for both bt first (gives ACT a head start),
            # then attn@v + output projection per bt.
            # tanh outputs land in one [128, 8*256] tile per (b, mc), with
            # heads at pair-order positions [0,4,1,5,2,6,3,7] so that a single
            # N=512 attn@v matmul covers a (p, p+4) head pair.
            pos = {h: i for i, h in enumerate((0, 4, 1, 5, 2, 6, 3, 7))}
            attn = [[None, None] for _ in range(2)]
            for b in range(2):
                for mc in range(2):
                    amc = apool.tile([128, H * 256], F32R, tag="attn")
                    attn[b][mc] = amc
                for h in range(H):
                    half = h % 2            # row half of q tile / PE array
                    grp = h // 4
                    # pick the k layout whose needed group sits in `half`
                    ksrc = ks[0] if (grp == half == 0 or grp == half == 1) \
                        else ks[1]
                    # rows of both operands at base partition 64*half
                    r0, r1 = 64 * half, 64 * (half + 1)
                    rhs = qs[h // 2][r0:r1, 256 * b:256 * (b + 1)]
                    for mc in range(2):
                        off = 256 * b + 128 * mc
                        ps = psB.tile([128, 256], F32, tag="psB")
                        mm(ps[:], ksrc[r0:r1, off:off + 128],
                                         rhs, start=True, stop=True)
                        o0 = 256 * pos[h]
                        nc.scalar.activation(
                            attn[b][mc][:, o0:o0 + 256], ps[:],
                            mybir.ActivationFunctionType.Tanh, scale=SCALE)

            for b in range(2):
                pairs = []
                for p in range(4):
                    # one N=512 matmul per (pair, mc): full-v lhsT against the
                    # [attn_p | attn_p+4] slice.  Valid blocks: head p (g0) in
                    # rows 0:64 cols 0:256, head p+4 (g1) in rows 64:128 cols
                    # 256:512; the sgr@v add picks exactly those blocks.
                    ps = psA.tile([128, 512], F32, tag="psA")
                    for mc in range(2):
                        mm(ps[:], vs[b][mc][:],
                           attn[b][mc][:, 512 * p:512 * (p + 1)],
                           start=(mc == 0), stop=(mc == 1))
                    s = ppool.tile([128, 256], F32R, tag="pairs")
                    nc.vector.tensor_add(s[0:64, :], ps[0:64, 0:256],
                                         svs[b][0:64, :])
                    nc.vector.tensor_add(s[64:128, :], ps[64:128, 256:512],
                                         svs[b][64:128, :])
                    pairs.append(s)

                for tc_ in range(2):
                    ps = psA.tile([128, 512], F32, tag="psA")
                    for p in range(4):
                        mm(ps[:],
                                         pairs[p][:, 128 * tc_:128 * (tc_ + 1)],
                                         wo[p][:], start=(p == 0), stop=(p == 3))
                    s = ypool.tile([128, 512], F32, tag="ys")
                    nc.vector.tensor_copy(s[:], ps[:])
                    nc.sync.dma_start(
                        y[2 * it + b, 128 * tc_:128 * (tc_ + 1), :], s[:])
            state[it] = None

        # 1-deep software pipeline: stage A of iter i+1 is emitted (and thus
        # sits in the PE queue) before stage B of iter i, so projections of
        # the next pair overlap the tanh/attention tail of the current one.
        stage_a(0)
        for it in range(NPAIR):
            if it + 1 < NPAIR:
                stage_a(it + 1)
            stage_b(it)


def _get_runner():
    if "nc" not in _cached:
        _cached["nc"] = _build_nc()
    return _cached["nc"]


def _prep_inputs(x, sgr, Wq, Wk, Wv, Wo):
    x = np.ascontiguousarray(x, dtype=np.float32)
    xb = x.reshape(BT, N, C)
    # Wk with groups swapped, concatenated
    Wk = np.asarray(Wk, dtype=np.float32)
    Wkc = np.concatenate([Wk, np.concatenate([Wk[:, D:], Wk[:, :D]], axis=1)],
                         axis=1)
    # Wo rows permuted to pair order [h0,h4 | h1,h5 | h2,h6 | h3,h7]
    perm = np.concatenate(
        [np.r_[64 * p:64 * (p + 1), 64 * (p + 4):64 * (p + 5)]
         for p in range(4)])
    Wop = np.ascontiguousarray(np.asarray(Wo, dtype=np.float32)[perm, :])
    sgrT = np.ascontiguousarray(np.asarray(sgr, dtype=np.float32).T)
    Wq = np.ascontiguousarray(np.asarray(Wq, dtype=np.float32))
    Wv = np.ascontiguousarray(np.asarray(Wv, dtype=np.float32))

    in_maps = []
    for core in range(NCORES):
        xc = xb[PER_CORE * core: PER_CORE * (core + 1)]        # [16, 256, 512]
        xtc = xc.transpose(0, 2, 1)                            # [16, 512, 256]
        xarr = np.ascontiguousarray(
            xtc.reshape(NPAIR, 2, 4, 128, N)
               .transpose(0, 3, 2, 1, 4)
               .reshape(NPAIR, 128, 4, 512))
        in_maps.append({
            "xT": xarr, "sgrT": sgrT, "Wq": Wq, "Wkc": Wkc,
            "Wv": Wv, "Wop": Wop,
        })
    return in_maps


def _run(x, sgr, Wq, Wk, Wv, Wo, trace=False, tmpdir=None):
    nc = _get_runner()
    in_maps = _prep_inputs(x, sgr, Wq, Wk, Wv, Wo)
    res = run_bass_kernel_spmd(nc, in_maps, list(range(NCORES)), trace=trace,
                               tmpdir=tmpdir)
    outs = [res.results[i]["y"] for i in range(NCORES)]
    full = np.concatenate(outs, axis=0).reshape(B, T, N, C)
    return full, res


def kernel(x, sgr, Wq, Wk, Wv, Wo):
    out, _ = _run(x, sgr, Wq, Wk, Wv, Wo, trace=False)
    return out


# revision 10
# speedup vs baseline: 1.2740x; 1.2740x over previous
"""GQA attention (tanh-score + static bias, no softmax) on 8 trn2 cores.

Reference shapes: x [4,32,256,512], H=8 heads, G=2 kv groups, D=64, N=256.
Strategy: data-parallel over the 128 (b,t) pairs -> 16 per core, zero
collectives.  Per (b,t):
  q = x@Wq, k = x@Wk, v = x@Wv          (feature-major via host-transposed x)
  scores^T[m,n] = k_g @ q_h^T           (K=64 contraction, base-aligned)
  attn^T = tanh(scores^T * 0.125)       (ACT engine, scale fused)
  out_h^T = v_g^T @ attn_h^T + (sgr v_g)^T   (sgr@v once per group, add fused
                                              into the PSUM->SBUF evacuation)
  y = out @ Wo                           (Wo host-permuted to match pair order)

Host-side prep (outside the HW kernel): x transposed to feature-major and
pre-tiled, sgr transposed, Wk concatenated with its group-swapped copy (so
every head's score matmul finds its K block at the right partition offset),
Wo row-permuted.
"""

import os
import sys

import numpy as np

for _p in ("/opt/trn_rl_repo",):
    if _p not in sys.path and os.path.isdir(_p):
        sys.path.insert(0, _p)

import concourse.bass as bass
import concourse.tile as tile
from concourse import bacc, mybir
from concourse.bass_utils import run_bass_kernel_spmd

F32 = mybir.dt.float32
F32R = mybir.dt.float32r

B, T, N, C = 4, 32, 256, 512
H, G, D = 8, 2, 64
NCORES = 8
BT = B * T                      # 128
PER_CORE = BT // NCORES         # 16
NPAIR = PER_CORE // 2           # 8 iterations of 2 (b,t) each
SCALE = D ** -0.5               # 0.125

_cached = {}


def _build_nc():
    """Build + lower the single-core SPMD program."""
    nc = bacc.Bacc("TRN2", target_bir_lowering=False, debug=False,
                   num_devices=NCORES)

    # DRAM I/O (per-core shard, host-side pre-arranged)
    # xarr[i, p, c, 256*b + n] = x[bt=2i+b, tok=n, cin=128c+p]
    xT = nc.dram_tensor("xT", [NPAIR, 128, 4, 512], F32R, kind="ExternalInput").ap()
    sgrT = nc.dram_tensor("sgrT", [N, N], F32R, kind="ExternalInput").ap()
    Wq = nc.dram_tensor("Wq", [C, C], F32R, kind="ExternalInput").ap()
    Wkc = nc.dram_tensor("Wkc", [C, 2 * G * D], F32R, kind="ExternalInput").ap()
    Wv = nc.dram_tensor("Wv", [C, G * D], F32R, kind="ExternalInput").ap()
    Wop = nc.dram_tensor("Wop", [C, C], F32R, kind="ExternalInput").ap()
    y = nc.dram_tensor("y", [PER_CORE, N, C], F32, kind="ExternalOutput").ap()

    with tile.TileContext(nc) as tc:
        _body(tc, xT, sgrT, Wq, Wkc, Wv, Wop, y)

    nc.compile()
    return nc


def _body(tc, xT, sgrT, Wq, Wkc, Wv, Wop, y):
    nc = tc.nc

    def mm(out, lhsT, rhs, **kw):
        # float32r streams 1 row/cycle (vs 4 for fp32) at free dim >= 256.
        # All lhsT/rhs tiles are allocated as F32R so their producers round.
        nc.tensor.matmul(out, lhsT, rhs, **kw)
    import contextlib
    ctx = contextlib.ExitStack()
    with ctx:
        consts = ctx.enter_context(tc.tile_pool(name="consts", bufs=1))
        xpool = ctx.enter_context(tc.tile_pool(name="xt", bufs=2))
        qpool = ctx.enter_context(tc.tile_pool(name="qs", bufs=8))
        kpool = ctx.enter_context(tc.tile_pool(name="ks", bufs=4))
        vpool = ctx.enter_context(tc.tile_pool(name="vs", bufs=8))
        svpool = ctx.enter_context(tc.tile_pool(name="svs", bufs=4))
        apool = ctx.enter_context(tc.tile_pool(name="attn", bufs=34))
        ppool = ctx.enter_context(tc.tile_pool(name="pairs", bufs=10))
        ypool = ctx.enter_context(tc.tile_pool(name="ys", bufs=6))
        psA = ctx.enter_context(
            tc.tile_pool(name="psA", bufs=3, space=bass.MemorySpace.PSUM))
        psB = ctx.enter_context(
            tc.tile_pool(name="psB", bufs=5, space=bass.MemorySpace.PSUM))

        # ---- resident constants ----
        wq = []
        wkc = []
        wv = []
        wo = []
        for c in range(4):
            t = consts.tile([128, 512], F32R, tag=f"wq{c}")
            nc.sync.dma_start(t[:], Wq[128 * c:128 * (c + 1), :])
            wq.append(t)
            t = consts.tile([128, 256], F32R, tag=f"wkc{c}")
            nc.sync.dma_start(t[:], Wkc[128 * c:128 * (c + 1), :])
            wkc.append(t)
            t = consts.tile([128, 128], F32R, tag=f"wv{c}")
            nc.sync.dma_start(t[:], Wv[128 * c:128 * (c + 1), :])
            wv.append(t)
            t = consts.tile([128, 512], F32R, tag=f"wo{c}")
            nc.sync.dma_start(t[:], Wop[128 * c:128 * (c + 1), :])
            wo.append(t)
        sgt = []
        for mc in range(2):
            t = consts.tile([128, 256], F32R, tag=f"sgt{mc}")
            nc.sync.dma_start(t[:], sgrT[128 * mc:128 * (mc + 1), :])
            sgt.append(t)

        # per-iteration state handed from stage A to stage B
        state = [None] * NPAIR

        def stage_a(it):
            xt = xpool.tile([128, 4, 512], F32R, tag="xt")
            nc.sync.dma_start(xt[:], xT[it])

            # q projection: feature-major q^T, couts 128j..128j+127,
            # free = 512 (two bt's 256 tokens each)
            qs = []
            for j in range(4):
                ps = psA.tile([128, 512], F32, tag="psA")
                for c in range(4):
                    mm(ps[:], wq[c][:, 128 * j:128 * (j + 1)],
                                     xt[:, c, :], start=(c == 0), stop=(c == 3))
                s = qpool.tile([128, 512], F32R, tag="qs")
                nc.vector.tensor_copy(s[:], ps[:])
                qs.append(s)

            # k projections: k1 = [g0;g1] rows, k2 = [g1;g0] rows
            ks = []
            for jj in range(2):
                ps = psA.tile([128, 512], F32, tag="psA")
                for c in range(4):
                    mm(ps[:], wkc[c][:, 128 * jj:128 * (jj + 1)],
                                     xt[:, c, :], start=(c == 0), stop=(c == 3))
                s = kpool.tile([128, 512], F32R, tag="ks")
                nc.vector.tensor_copy(s[:], ps[:])
                ks.append(s)

            # v token-major: [tok 128, 128 (g0 d | g1 d)] per (b, tok-chunk)
            vs = [[None, None], [None, None]]
            for b in range(2):
                for mc in range(2):
                    ps = psB.tile([128, 128], F32, tag="psB")
                    off = 256 * b + 128 * mc
                    for c in range(4):
                        mm(ps[:], xt[:, c, off:off + 128],
                                         wv[c][:], start=(c == 0), stop=(c == 3))
                    s = vpool.tile([128, 128], F32R, tag="vs")
                    nc.vector.tensor_copy(s[:], ps[:])
                    vs[b][mc] = s

            # sgr @ v, both groups at once: lhsT = full v tile [m,128] so
            # out rows 0:64 = (sgr v_g0)^T, rows 64:128 = (sgr v_g1)^T and the
            # PSUM dst stays at partition base 0 (fp32r ISA constraint).
            svs = []
            for b in range(2):
                ps = psB.tile([128, 256], F32, tag="psB")
                for mc in range(2):
                    mm(ps[:], vs[b][mc][:], sgt[mc][:],
                       start=(mc == 0), stop=(mc == 1))
                s = svpool.tile([128, 256], F32, tag="svs")
                nc.vector.tensor_copy(s[:], ps[:])
                svs.append(s)

            state[it] = (qs, ks, vs, svs)

        def stage_b(it):
            qs, ks, vs, svs = state[it]
            # scores + tanh for both bt first (gives ACT a head start),
            # then attn@v + output projection per bt.
            attn = [[[None, None] for _ in range(H)] for _ in range(2)]
            for b in range(2):
                for h in range(H):
                    half = h % 2            # row half of q tile / PE array
                    grp = h // 4
                    # pick the k layout whose needed group sits in `half`
                    ksrc = ks[0] if (grp == half == 0 or grp == half == 1) \
                        else ks[1]
                    # rows of both operands at base partition 64*half
                    r0, r1 = 64 * half, 64 * (half + 1)
                    rhs = qs[h // 2][r0:r1, 256 * b:256 * (b + 1)]
                    for mc in range(2):
                        off = 256 * b + 128 * mc
                        ps = psB.tile([128, 256], F32, tag="psB")
                        mm(ps[:], ksrc[r0:r1, off:off + 128],
                                         rhs, start=True, stop=True)
                        a = apool.tile([128, 256], F32R, tag="attn")
                        nc.scalar.activation(
                            a[:], ps[:], mybir.ActivationFunctionType.Tanh,
                            scale=SCALE)
                        attn[b][h][mc] = a

            for b in range(2):
                pairs = []
                for p in range(4):
                    # full-v lhsT: head h's matmul yields (attn_h @ v_g0)^T in
                    # rows 0:64 and (attn_h @ v_g1)^T in rows 64:128; keep the
                    # half belonging to h's group.  PSUM dst base stays 0.
                    psl = psB.tile([128, 256], F32, tag="psB")
                    psh = psB.tile([128, 256], F32, tag="psB")
                    for mc in range(2):
                        mm(psl[:], vs[b][mc][:], attn[b][p][mc][:],
                           start=(mc == 0), stop=(mc == 1))
                    for mc in range(2):
                        mm(psh[:], vs[b][mc][:], attn[b][p + 4][mc][:],
                           start=(mc == 0), stop=(mc == 1))
                    s = ppool.tile([128, 256], F32R, tag="pairs")
                    nc.vector.tensor_add(s[0:64, :], psl[0:64, :],
                                         svs[b][0:64, :])
                    nc.vector.tensor_add(s[64:128, :], psh[64:128, :],
                                         svs[b][64:128, :])
                    pairs.append(s)

                for tc_ in range(2):
                    ps = psA.tile([128, 512], F32, tag="psA")
                    for p in range(4):
                        mm(ps[:],
                                         pairs[p][:, 128 * tc_:128 * (tc_ + 1)],
                                         wo[p][:], start=(p == 0), stop=(p == 3))
                    s = ypool.tile([128, 512], F32, tag="ys")
                    nc.vector.tensor_copy(s[:], ps[:])
                    nc.sync.dma_start(
                        y[2 * it + b, 128 * tc_:128 * (tc_ + 1), :], s[:])
            state[it] = None

        # 1-deep software pipeline: stage A of iter i+1 is emitted (and thus
        # sits in the PE queue) before stage B of iter i, so projections of
        # the next pair overlap the tanh/attention tail of the current one.
        stage_a(0)
        for it in range(NPAIR):
            if it + 1 < NPAIR:
                stage_a(it + 1)
            stage_b(it)


def _get_runner():
    if "nc" not in _cached:
        _cached["nc"] = _build_nc()
    return _cached["nc"]


def _prep_inputs(x, sgr, Wq, Wk, Wv, Wo):
    x = np.ascontiguousarray(x, dtype=np.float32)
    xb = x.reshape(BT, N, C)
    # Wk with groups swapped, concatenated
    Wk = np.asarray(Wk, dtype=np.float32)
    Wkc = np.concatenate([Wk, np.concatenate([Wk[:, D:], Wk[:, :D]], axis=1)],
                         axis=1)
    # Wo rows permuted to pair order [h0,h4 | h1,h5 | h2,h6 | h3,h7]
    perm = np.concatenate(
        [np.r_[64 * p:64 * (p + 1), 64 * (p + 4):64 * (p + 5)]
         for p in range(4)])
    Wop = np.ascontiguousarray(np.asarray(Wo, dtype=np.float32)[perm, :])
    sgrT = np.ascontiguousarray(np.asarray(sgr, dtype=np.float32).T)
    Wq = np.ascontiguousarray(np.asarray(Wq, dtype=np.float32))
    Wv = np.ascontiguousarray(np.asarray(Wv, dtype=np.float32))

    in_maps = []
    for core in range(NCORES):
        xc = xb[PER_CORE * core: PER_CORE * (core + 1)]        # [16, 256, 512]
        xtc = xc.transpose(0, 2, 1)                            # [16, 512, 256]
        xarr = np.ascontiguousarray(
            xtc.reshape(NPAIR, 2, 4, 128, N)
               .transpose(0, 3, 2, 1, 4)
               .reshape(NPAIR, 128, 4, 512))
        in_maps.append({
            "xT": xarr, "sgrT": sgrT, "Wq": Wq, "Wkc": Wkc,
            "Wv": Wv, "Wop": Wop,
        })
    return in_maps


def _run(x, sgr, Wq, Wk, Wv, Wo, trace=False, tmpdir=None):
    nc = _get_runner()
    in_maps = _prep_inputs(x, sgr, Wq, Wk, Wv, Wo)
    res = run_bass_kernel_spmd(nc, in_maps, list(range(NCORES)), trace=trace,
                               tmpdir=tmpdir)
    outs = [res.results[i]["y"] for i in range(NCORES)]
    full = np.concatenate(outs, axis=0).reshape(B, T, N, C)
    return full, res


def kernel(x, sgr, Wq, Wk, Wv, Wo):
    out, _ = _run(x, sgr, Wq, Wk, Wv, Wo, trace=False)
    return out
